# revision 7
# baseline (speedup 1.0000x reference)
"""AFNB (asymmetric fusion non-local block) Trainium2 kernel, 8-core SPMD.

Data-parallel over batch: 16 batches -> 2 per core, no collectives.

Algebra (per batch, softmax over the QUERY axis allows folding):
  theta = w_theta @ y        [IC, N]   (split2: bf16 weights, y = yh+yl bf16)
  th_spp = SPP(theta)        [IC, S]   (exact fp32 max-pool)
  g    = w_g @ y             [IC, N]   (bf16)
  g_spp = SPP(g)             [IC, S]   (bf16)
  M1T  = w_phi^T @ th_spp    [C, S]    (fp32)
  scoresT = M1T^T @ x        [S, N]    (split2: bf16 M1, x = xh+xl)
  e    = exp(scoresT - rowmax)         (ACT, accum -> denom)
  M2T  = (g_spp^T @ w_mask^T) / denom  [S, C] (bf16)
  out  = M2T^T @ e + x       [C, N]
"""

import numpy as np
import ml_dtypes

import concourse.bass as bass
import concourse.tile as tile
from concourse import bacc, mybir
from concourse.ap import AP
from concourse.bass_utils import run_bass_kernel_spmd

F32 = mybir.dt.float32
BF16 = mybir.dt.bfloat16
AX = mybir.AxisListType
OP = mybir.AluOpType

B, C, HH, WW = 16, 512, 64, 64
N = HH * WW
IC = 256
NCORES = 8
NB = B // NCORES  # batches per core
P = 128
KC = C // P   # 4 contraction chunks over C
MI = IC // P  # 2 chunks over IC
NN = 8        # n-chunks
NT = N // NN  # 512
OUT_SIZES = [1, 3, 6, 8]
S = sum(o * o for o in OUT_SIZES)  # 110
SPP_OFF = {1: 0, 3: 1, 6: 10, 8: 46}


def _bounds(n, o):
    return [((i * n) // o, ((i + 1) * n + o - 1) // o) for i in range(o)]


def _atoms():
    bs = set()
    for o in OUT_SIZES:
        for s, e in _bounds(HH, o):
            bs.add(s); bs.add(e)
    bs = sorted(bs)
    return [(bs[i], bs[i + 1]) for i in range(len(bs) - 1)]


ATOMS = _atoms()          # 16 atomic row intervals
NA = len(ATOMS)


def _bin_atom_ranges(o):
    """For each bin of size-o pooling: (first_atom_idx, last_atom_idx_excl)."""
    out = []
    for s, e in _bounds(HH, o):
        a0 = next(i for i, (as_, _) in enumerate(ATOMS) if as_ == s)
        a1 = next(i for i, (_, ae) in enumerate(ATOMS) if ae == e) + 1
        out.append((a0, a1))
    return out


def _grouped(items):
    """Group indices j of (start, length) items into classes {j = r mod m} where
    each class has constant length and arithmetic starts. Returns list of
    (j0, m, cnt, start0, dstart, length)."""
    n = len(items)
    for m in range(1, n + 1):
        groups = []
        ok = True
        for r in range(m):
            js = list(range(r, n, m))
            lens = {items[j][1] for j in js}
            if len(lens) != 1:
                ok = False; break
            starts = [items[j][0] for j in js]
            d = starts[1] - starts[0] if len(starts) > 1 else 0
            if any(starts[i + 1] - starts[i] != d for i in range(len(starts) - 1)):
                ok = False; break
            groups.append((r, m, len(js), starts[0], d, lens.pop()))
        if ok:
            return groups
    raise AssertionError


ROW_GROUPS = {o: _grouped([(a0, a1 - a0) for a0, a1 in _bin_atom_ranges(o)])
              for o in OUT_SIZES}
COL_GROUPS = {o: _grouped([(s, e - s) for s, e in _bounds(WW, o)])
              for o in OUT_SIZES}
RB_OFF = {}  # row-bin output offset (units of 64 cols) per o
_off = 0
for _o in OUT_SIZES:
    RB_OFF[_o] = _off
    _off += _o
RB_TOT = _off  # 18


def mk(ap_base, off_elems, dims):
    """Custom free-dim AP over a tile's base AP: dims = [(step, count), ...]."""
    part = list(ap_base.ap[0])
    return AP(tensor=ap_base.tensor, offset=ap_base.offset + off_elems,
              ap=[part] + [[s, c] for s, c in dims])


def build():
    nc = bacc.Bacc("TRN2", target_bir_lowering=False, debug=False,
                   num_devices=NCORES)
    x_ext = nc.declare_dram_parameter("x", [NB, C, N], F32, isOutput=False)
    y_ext = nc.declare_dram_parameter("y", [NB, C, N], F32, isOutput=False)
    wth_ext = nc.declare_dram_parameter("wthT", [C, IC], BF16, isOutput=False)
    wg_ext = nc.declare_dram_parameter("wgT", [C, IC], BF16, isOutput=False)
    wphi_ext = nc.declare_dram_parameter("wphi", [IC, C], F32, isOutput=False)
    wmk_ext = nc.declare_dram_parameter("wmkT", [IC, C], BF16, isOutput=False)
    out_ext = nc.declare_dram_parameter("out", [NB, C, N], F32, isOutput=True)

    with tile.TileContext(nc) as tc:
        with (
            tc.tile_pool(name="w", bufs=1) as wp,
            tc.tile_pool(name="io32", bufs=6) as iop,
            tc.tile_pool(name="hlc", bufs=10) as hlp,
            tc.tile_pool(name="pool", bufs=1) as pp,
            tc.tile_pool(name="attn", bufs=2) as ap_,
            tc.tile_pool(name="ostg", bufs=6) as osp,
            tc.tile_pool(name="psc", bufs=2, space="PSUM") as ps_conv,
            tc.tile_pool(name="psa", bufs=3, space="PSUM") as ps_attn,
            tc.tile_pool(name="pss", bufs=1, space="PSUM") as ps_small,
        ):
            # ---- weights (resident) ----
            wth_t = [wp.tile([P, IC], BF16, tag=f"wth{k}", name=f"wth{k}") for k in range(KC)]
            wg_t = [wp.tile([P, IC], BF16, tag=f"wg{k}", name=f"wg{k}") for k in range(KC)]
            wphi_t = [wp.tile([P, C], F32, tag=f"wphi{k}", name=f"wphi{k}") for k in range(MI)]
            wmk_t = [wp.tile([P, C], BF16, tag=f"wmk{k}", name=f"wmk{k}") for k in range(MI)]
            for k in range(KC):
                nc.sync.dma_start(wth_t[k][:], wth_ext[k * P:(k + 1) * P, :])
                nc.sync.dma_start(wg_t[k][:], wg_ext[k * P:(k + 1) * P, :])
            for k in range(MI):
                nc.sync.dma_start(wphi_t[k][:], wphi_ext[k * P:(k + 1) * P, :])
                nc.sync.dma_start(wmk_t[k][:], wmk_ext[k * P:(k + 1) * P, :])

            ps = (ps_conv, ps_attn, ps_small)
            ctxs = [BatchCtx(b) for b in range(NB)]
            A = dict(nc=nc, x_ext=x_ext, y_ext=y_ext, out_ext=out_ext,
                     wth_t=wth_t, wg_t=wg_t, wphi_t=wphi_t, wmk_t=wmk_t,
                     iop=iop, hlp=hlp, pp=pp, osp=osp, ps=ps)
            phase_conv(ctxs[0], **A)
            phase_binpool_m1(ctxs[0], **A)
            phase_scores(ctxs[0], **A)
            phase_softmax_m2(ctxs[0], **A)
            phase_conv(ctxs[1], **A)
            phase_binpool_m1(ctxs[1], **A)
            phase_mask_out(ctxs[0], **A)
            phase_scores(ctxs[1], **A)
            phase_softmax_m2(ctxs[1], **A)
            phase_mask_out(ctxs[1], **A)

    nc.compile()
    return nc


class BatchCtx:
    def __init__(self, b):
        self.b = b


def spp_reduce_from_psum(nc, pt, nn, ratom):
    """Stage R: row-atom max-pool from a conv psum chunk [128,512] (8 image
    rows) into this tensor's ratom tile."""
    base = pt[:]
    for ai, (s, e) in enumerate(ATOMS):
        if s >= 8 * nn and e <= 8 * (nn + 1):
            ls = s - 8 * nn
            src = mk(base, ls * WW, [(1, WW), (WW, e - s)])
            dst = mk(ratom[:], ai * WW, [(1, WW)])
            nc.vector.reduce_max(dst, src, axis=AX.X)


def spp_bins(nc, ratom, rb, spp):
    """B1 (row bins from atoms) + B2 (col bins, strided uniform groups)."""
    for o in OUT_SIZES:
        for (r, m, cnt, a0, da, ln) in ROW_GROUPS[o]:
            src = mk(ratom[:], a0 * WW, [(da * WW, cnt), (1, WW), (WW, ln)])
            dst = mk(rb[:], (RB_OFF[o] + r) * WW, [(m * WW, cnt), (1, WW)])
            nc.vector.reduce_max(dst, src, axis=AX.X)
        for (r, m, cnt, s0, ds, ln) in COL_GROUPS[o]:
            src = mk(rb[:], RB_OFF[o] * WW + s0, [(WW, o), (ds, cnt), (1, ln)])
            dst = mk(spp[:], SPP_OFF[o] + r, [(o, o), (m, cnt)])
            nc.vector.reduce_max(dst, src, axis=AX.X)


def phase_conv(cx, nc, y_ext, iop, hlp, pp, ps, wth_t, wg_t, **_):
    b = cx.b
    ps_conv, _, _ = ps
    cx.y_t = []
    for k in range(KC):
        t = iop.tile([P, N], F32, tag="io32", name=f"yt_{b}_{k}")
        nc.sync.dma_start(t[:, 0:N // 2], y_ext[b, k * P:(k + 1) * P, 0:N // 2])
        nc.sync.dma_start(t[:, N // 2:N], y_ext[b, k * P:(k + 1) * P, N // 2:N])
        cx.y_t.append(t)
    cx.rat_th = [pp.tile([P, NA * WW], F32, tag=f"rath{mi}", name=f"rath{mi}_{b}")
                 for mi in range(MI)]
    cx.rat_g = [pp.tile([P, NA * WW], BF16, tag=f"ratg{mi}", name=f"ratg{mi}_{b}")
                for mi in range(MI)]
    for nn in range(NN):
        yh_c, yl_c = [], []
        for k in range(KC):
            ysl = cx.y_t[k][:, nn * NT:(nn + 1) * NT]
            h = hlp.tile([P, NT], BF16, tag="hlc", name=f"h_{b}_{nn}_{k}")
            if k % 2 == 0:
                nc.scalar.copy(h[:], ysl)
            else:
                nc.gpsimd.tensor_copy(h[:], ysl)
            l = hlp.tile([P, NT], BF16, tag="hlc", name=f"l_{b}_{nn}_{k}")
            nc.gpsimd.tensor_sub(l[:], ysl, h[:])
            yh_c.append(h); yl_c.append(l)
        for mi in range(MI):
            pt = ps_conv.tile([P, NT], F32, tag=f"conv{mi}", name=f"pth{mi}_{b}_{nn}")
            for k in range(KC):
                nc.tensor.matmul(pt[:], wth_t[k][:, mi * P:(mi + 1) * P],
                                 yh_c[k][:], start=(k == 0), stop=False)
            for k in range(KC):
                nc.tensor.matmul(pt[:], wth_t[k][:, mi * P:(mi + 1) * P],
                                 yl_c[k][:], start=False, stop=(k == KC - 1))
            spp_reduce_from_psum(nc, pt, nn, cx.rat_th[mi])
        for mi in range(MI):
            pg = ps_conv.tile([P, NT], F32, tag=f"conv{mi}", name=f"pg{mi}_{b}_{nn}")
            for k in range(KC):
                nc.tensor.matmul(pg[:], wg_t[k][:, mi * P:(mi + 1) * P],
                                 yh_c[k][:], start=(k == 0), stop=(k == KC - 1))
            spp_reduce_from_psum(nc, pg, nn, cx.rat_g[mi])


def phase_binpool_m1(cx, nc, pp, ps, wphi_t, **_):
    b = cx.b
    _, _, ps_small = ps
    cx.spp = []
    cx.g_bf = []
    for mi in range(MI):
        rbt = pp.tile([P, RB_TOT * WW], F32, tag=f"rbth{mi}", name=f"rbth{mi}_{b}")
        sppt = pp.tile([P, S], F32, tag=f"sppth{mi}", name=f"sppth{mi}_{b}")
        spp_bins(nc, cx.rat_th[mi], rbt, sppt)
        cx.spp.append(sppt)
        rbg = pp.tile([P, RB_TOT * WW], BF16, tag=f"rbg{mi}", name=f"rbg{mi}_{b}")
        gbf = pp.tile([P, S], BF16, tag=f"gbf{mi}", name=f"gbf{mi}_{b}")
        spp_bins(nc, cx.rat_g[mi], rbg, gbf)
        cx.g_bf.append(gbf)
    cx.m1_bf = []
    for mc in range(KC):
        pm = ps_small.tile([P, S], F32, tag="psmall", name=f"pm1_{b}_{mc}")
        for k in range(MI):
            nc.tensor.matmul(pm[:], wphi_t[k][:, mc * P:(mc + 1) * P],
                             cx.spp[k][:], start=(k == 0), stop=(k == MI - 1))
        m = pp.tile([P, S], BF16, tag=f"m1_{mc}", name=f"m1b_{b}_{mc}")
        nc.scalar.copy(m[:], pm[:])
        cx.m1_bf.append(m)


def phase_scores(cx, nc, x_ext, iop, hlp, pp, ps, **_):
    b = cx.b
    _, ps_attn, _ = ps
    cx.x_t = []
    for k in range(KC):
        t = iop.tile([P, N], F32, tag="io32", name=f"xt_{b}_{k}")
        nc.sync.dma_start(t[:, 0:N // 2], x_ext[b, k * P:(k + 1) * P, 0:N // 2])
        nc.sync.dma_start(t[:, N // 2:N], x_ext[b, k * P:(k + 1) * P, N // 2:N])
        cx.x_t.append(t)
    cx.sc_sb = pp.tile([S, N], F32, tag="scsb", name=f"scsb_{b}")
    cx.gm = pp.tile([S, 16], F32, tag="gm", name=f"gm_{b}")
    for nn in range(NN):
        xh_c = []
        for k in range(KC):
            xsl = cx.x_t[k][:, nn * NT:(nn + 1) * NT]
            h = hlp.tile([P, NT], BF16, tag="hlc", name=f"xh_{b}_{nn}_{k}")
            if k % 2 == 0:
                nc.scalar.copy(h[:], xsl)
            else:
                nc.gpsimd.tensor_copy(h[:], xsl)
            xh_c.append(h)
        psc = ps_attn.tile([S, NT], F32, tag="pattn", name=f"psc_{b}_{nn}")
        for k in range(KC):
            nc.tensor.matmul(psc[:], cx.m1_bf[k][:], xh_c[k][:],
                             start=(k == 0), stop=(k == KC - 1))
        nc.vector.reduce_max(cx.gm[:, nn:nn + 1], psc[:], axis=AX.X)
        nc.scalar.copy(cx.sc_sb[:, nn * NT:(nn + 1) * NT], psc[:])


def phase_softmax_m2(cx, nc, pp, ps, wmk_t, **_):
    b = cx.b
    _, _, ps_small = ps
    gmax = pp.tile([S, 1], F32, tag="gmax", name=f"gmax_{b}")
    nc.vector.reduce_max(gmax[:], cx.gm[:, 0:NN], axis=AX.X)
    ngmax = pp.tile([S, 1], F32, tag="ngmax", name=f"ngmax_{b}")
    nc.vector.tensor_scalar_mul(ngmax[:], gmax[:], -1.0)
    cx.e_bf = pp.tile([S, N], BF16, tag="ebf", name=f"ebf_{b}")
    dsum = pp.tile([S, 1], F32, tag="dsum", name=f"dsum_{b}")
    nc.scalar.activation(cx.e_bf[:], cx.sc_sb[:], mybir.ActivationFunctionType.Exp,
                         bias=ngmax[:], scale=1.0, accum_out=dsum[:])
    rden = pp.tile([S, 1], F32, tag="rden", name=f"rden_{b}")
    nc.vector.reciprocal(rden[:], dsum[:])
    pm2 = ps_small.tile([S, C], F32, tag="psmall", name=f"pm2_{b}")
    for k in range(MI):
        nc.tensor.matmul(pm2[:], cx.g_bf[k][:], wmk_t[k][:],
                         start=(k == 0), stop=(k == MI - 1))
    cx.m2_bf = pp.tile([S, C], BF16, tag="m2", name=f"m2_{b}")
    nc.vector.tensor_scalar_mul(cx.m2_bf[:], pm2[:], rden[:])


def phase_mask_out(cx, nc, out_ext, osp, ps, **_):
    b = cx.b
    _, ps_attn, _ = ps
    for mc in range(KC):
        for nn in range(NN):
            pk = ps_attn.tile([P, NT], F32, tag="pattn", name=f"pk_{b}_{mc}_{nn}")
            nc.tensor.matmul(pk[:], cx.m2_bf[:, mc * P:(mc + 1) * P],
                             cx.e_bf[:, nn * NT:(nn + 1) * NT],
                             start=True, stop=True)
            xsl = cx.x_t[mc][:, nn * NT:(nn + 1) * NT]
            if (mc + nn) % 2 == 0:
                nc.vector.tensor_add(xsl, pk[:], xsl)
                nc.sync.dma_start(
                    out_ext[b, mc * P:(mc + 1) * P, nn * NT:(nn + 1) * NT], xsl)
            else:
                o = osp.tile([P, NT], F32, tag="ostg", name=f"ost_{b}_{mc}_{nn}")
                nc.scalar.copy(o[:], pk[:])
                nc.gpsimd.tensor_add(o[:], o[:], xsl)
                nc.sync.dma_start(
                    out_ext[b, mc * P:(mc + 1) * P, nn * NT:(nn + 1) * NT], o[:])


_NC_CACHE = {}


def _get_nc():
    if "nc" not in _NC_CACHE:
        _NC_CACHE["nc"] = build()
    return _NC_CACHE["nc"]


def kernel(x, y, w_phi, w_theta, w_g, w_mask):
    x = np.ascontiguousarray(np.asarray(x, dtype=np.float32))
    y = np.ascontiguousarray(np.asarray(y, dtype=np.float32))
    bf = ml_dtypes.bfloat16
    wthT = np.ascontiguousarray(np.asarray(w_theta, np.float32).T).astype(bf)
    wgT = np.ascontiguousarray(np.asarray(w_g, np.float32).T).astype(bf)
    wphi = np.ascontiguousarray(np.asarray(w_phi, np.float32))
    wmkT = np.ascontiguousarray(np.asarray(w_mask, np.float32).T).astype(bf)

    nc = _get_nc()
    in_maps = []
    for c in range(NCORES):
        sl = slice(c * NB, (c + 1) * NB)
        in_maps.append({
            "x": x[sl].reshape(NB, C, N),
            "y": y[sl].reshape(NB, C, N),
            "wthT": wthT, "wgT": wgT, "wphi": wphi, "wmkT": wmkT,
        })
    res = run_bass_kernel_spmd(nc, in_maps, core_ids=list(range(NCORES)))
    out = np.concatenate([r["out"].reshape(NB, C, HH, WW) for r in res.results],
                         axis=0)
    return out


# revision 8
# speedup vs baseline: 1.3965x; 1.3965x over previous
"""AFNB (asymmetric fusion non-local block) Trainium2 kernel, 8-core SPMD.

Data-parallel over batch: 16 batches -> 2 per core, no collectives.

Algebra (per batch, softmax over the QUERY axis allows folding):
  theta = w_theta @ y        [IC, N]   (split2: bf16 weights, y = yh+yl bf16)
  th_spp = SPP(theta)        [IC, S]   (exact fp32 max-pool)
  g    = w_g @ y             [IC, N]   (bf16)
  g_spp = SPP(g)             [IC, S]   (bf16)
  M1T  = w_phi^T @ th_spp    [C, S]    (fp32)
  scoresT = M1T^T @ x        [S, N]    (split2: bf16 M1, x = xh+xl)
  e    = exp(scoresT - rowmax)         (ACT, accum -> denom)
  M2T  = (g_spp^T @ w_mask^T) / denom  [S, C] (bf16)
  out  = M2T^T @ e + x       [C, N]
"""

import numpy as np
import ml_dtypes

import concourse.bass as bass
import concourse.tile as tile
from concourse import bacc, mybir
from concourse.ap import AP
from concourse.bass_utils import run_bass_kernel_spmd

F32 = mybir.dt.float32
BF16 = mybir.dt.bfloat16
AX = mybir.AxisListType
OP = mybir.AluOpType

B, C, HH, WW = 16, 512, 64, 64
N = HH * WW
IC = 256
NCORES = 8
NB = B // NCORES  # batches per core
P = 128
KC = C // P   # 4 contraction chunks over C
MI = IC // P  # 2 chunks over IC
NN = 8        # n-chunks
NT = N // NN  # 512
OUT_SIZES = [1, 3, 6, 8]
S = sum(o * o for o in OUT_SIZES)  # 110
SPP_OFF = {1: 0, 3: 1, 6: 10, 8: 46}


def _bounds(n, o):
    return [((i * n) // o, ((i + 1) * n + o - 1) // o) for i in range(o)]


def _atoms():
    bs = set()
    for o in OUT_SIZES:
        for s, e in _bounds(HH, o):
            bs.add(s); bs.add(e)
    bs = sorted(bs)
    return [(bs[i], bs[i + 1]) for i in range(len(bs) - 1)]


ATOMS = _atoms()          # 16 atomic row intervals
NA = len(ATOMS)


def _bin_atom_ranges(o):
    """For each bin of size-o pooling: (first_atom_idx, last_atom_idx_excl)."""
    out = []
    for s, e in _bounds(HH, o):
        a0 = next(i for i, (as_, _) in enumerate(ATOMS) if as_ == s)
        a1 = next(i for i, (_, ae) in enumerate(ATOMS) if ae == e) + 1
        out.append((a0, a1))
    return out


def _grouped(items):
    """Group indices j of (start, length) items into classes {j = r mod m} where
    each class has constant length and arithmetic starts. Returns list of
    (j0, m, cnt, start0, dstart, length)."""
    n = len(items)
    for m in range(1, n + 1):
        groups = []
        ok = True
        for r in range(m):
            js = list(range(r, n, m))
            lens = {items[j][1] for j in js}
            if len(lens) != 1:
                ok = False; break
            starts = [items[j][0] for j in js]
            d = starts[1] - starts[0] if len(starts) > 1 else 0
            if any(starts[i + 1] - starts[i] != d for i in range(len(starts) - 1)):
                ok = False; break
            groups.append((r, m, len(js), starts[0], d, lens.pop()))
        if ok:
            return groups
    raise AssertionError


ROW_GROUPS = {o: _grouped([(a0, a1 - a0) for a0, a1 in _bin_atom_ranges(o)])
              for o in OUT_SIZES}
COL_GROUPS = {o: _grouped([(s, e - s) for s, e in _bounds(WW, o)])
              for o in OUT_SIZES}
RB_OFF = {}  # row-bin output offset (units of 64 cols) per o
_off = 0
for _o in OUT_SIZES:
    RB_OFF[_o] = _off
    _off += _o
RB_TOT = _off  # 18


def mk(ap_base, off_elems, dims):
    """Custom free-dim AP over a tile's base AP: dims = [(step, count), ...]."""
    part = list(ap_base.ap[0])
    return AP(tensor=ap_base.tensor, offset=ap_base.offset + off_elems,
              ap=[part] + [[s, c] for s, c in dims])


def build():
    nc = bacc.Bacc("TRN2", target_bir_lowering=False, debug=False,
                   num_devices=NCORES)
    x_ext = nc.declare_dram_parameter("x", [NB, C, N], F32, isOutput=False)
    y_ext = nc.declare_dram_parameter("y", [NB, C, N], F32, isOutput=False)
    wth_ext = nc.declare_dram_parameter("wthT", [C, IC], BF16, isOutput=False)
    wg_ext = nc.declare_dram_parameter("wgT", [C, IC], BF16, isOutput=False)
    wphi_ext = nc.declare_dram_parameter("wphi", [IC, C], F32, isOutput=False)
    wmk_ext = nc.declare_dram_parameter("wmkT", [IC, C], BF16, isOutput=False)
    out_ext = nc.declare_dram_parameter("out", [NB, C, N], F32, isOutput=True)

    with tile.TileContext(nc) as tc:
        with (
            tc.tile_pool(name="w", bufs=1) as wp,
            tc.tile_pool(name="io32", bufs=6) as iop,
            tc.tile_pool(name="hlc", bufs=10) as hlp,
            tc.tile_pool(name="pool", bufs=1) as pp,
            tc.tile_pool(name="attn", bufs=2) as ap_,
            tc.tile_pool(name="ostg", bufs=6) as osp,
            tc.tile_pool(name="psc", bufs=2, space="PSUM") as ps_conv,
            tc.tile_pool(name="psa", bufs=3, space="PSUM") as ps_attn,
            tc.tile_pool(name="pss", bufs=1, space="PSUM") as ps_small,
        ):
            # ---- weights (resident) ----
            wth_t = [wp.tile([P, IC], BF16, tag=f"wth{k}", name=f"wth{k}") for k in range(KC)]
            wg_t = [wp.tile([P, IC], BF16, tag=f"wg{k}", name=f"wg{k}") for k in range(KC)]
            wphi_t = [wp.tile([P, C], F32, tag=f"wphi{k}", name=f"wphi{k}") for k in range(MI)]
            wmk_t = [wp.tile([P, C], BF16, tag=f"wmk{k}", name=f"wmk{k}") for k in range(MI)]
            for k in range(KC):
                nc.sync.dma_start(wth_t[k][:], wth_ext[k * P:(k + 1) * P, :])
                nc.sync.dma_start(wg_t[k][:], wg_ext[k * P:(k + 1) * P, :])
            for k in range(MI):
                nc.sync.dma_start(wphi_t[k][:], wphi_ext[k * P:(k + 1) * P, :])
                nc.sync.dma_start(wmk_t[k][:], wmk_ext[k * P:(k + 1) * P, :])

            ps = (ps_conv, ps_attn, ps_small)
            ctxs = [BatchCtx(b) for b in range(NB)]
            A = dict(nc=nc, x_ext=x_ext, y_ext=y_ext, out_ext=out_ext,
                     wth_t=wth_t, wg_t=wg_t, wphi_t=wphi_t, wmk_t=wmk_t,
                     iop=iop, hlp=hlp, pp=pp, osp=osp, ps=ps)
            phase_conv(ctxs[0], **A)
            phase_binpool_m1(ctxs[0], **A)
            phase_scores(ctxs[0], **A)
            phase_softmax_m2(ctxs[0], **A)
            phase_conv(ctxs[1], **A)
            phase_binpool_m1(ctxs[1], **A)
            phase_mask_out(ctxs[0], **A)
            phase_scores(ctxs[1], **A)
            phase_softmax_m2(ctxs[1], **A)
            phase_mask_out(ctxs[1], **A)

    nc.compile()
    return nc


class BatchCtx:
    def __init__(self, b):
        self.b = b


def spp_reduce_from_psum(nc, pt, nn, ratom):
    """Stage R: row-atom max-pool from a conv psum chunk [128,512] (8 image
    rows) into this tensor's ratom tile."""
    base = pt[:]
    for ai, (s, e) in enumerate(ATOMS):
        if s >= 8 * nn and e <= 8 * (nn + 1):
            ls = s - 8 * nn
            src = mk(base, ls * WW, [(1, WW), (WW, e - s)])
            dst = mk(ratom[:], ai * WW, [(1, WW)])
            nc.vector.reduce_max(dst, src, axis=AX.X)


def spp_bins(nc, ratom, rb, spp):
    """B1 (row bins from atoms) + B2 (col bins, strided uniform groups)."""
    for o in OUT_SIZES:
        for (r, m, cnt, a0, da, ln) in ROW_GROUPS[o]:
            src = mk(ratom[:], a0 * WW, [(da * WW, cnt), (1, WW), (WW, ln)])
            dst = mk(rb[:], (RB_OFF[o] + r) * WW, [(m * WW, cnt), (1, WW)])
            nc.vector.reduce_max(dst, src, axis=AX.X)
        for (r, m, cnt, s0, ds, ln) in COL_GROUPS[o]:
            src = mk(rb[:], RB_OFF[o] * WW + s0, [(WW, o), (ds, cnt), (1, ln)])
            dst = mk(spp[:], SPP_OFF[o] + r, [(o, o), (m, cnt)])
            nc.vector.reduce_max(dst, src, axis=AX.X)


def phase_conv(cx, nc, y_ext, iop, hlp, pp, ps, wth_t, wg_t, **_):
    b = cx.b
    ps_conv, _, _ = ps
    cx.y_t = []
    for k in range(KC):
        t = iop.tile([P, N], F32, tag="io32", name=f"yt_{b}_{k}")
        nc.sync.dma_start(t[:, 0:N // 2], y_ext[b, k * P:(k + 1) * P, 0:N // 2])
        nc.sync.dma_start(t[:, N // 2:N], y_ext[b, k * P:(k + 1) * P, N // 2:N])
        cx.y_t.append(t)
    cx.rat_th = [pp.tile([P, NA * WW], F32, tag=f"rath{mi}", name=f"rath{mi}_{b}")
                 for mi in range(MI)]
    cx.rat_g = [pp.tile([P, NA * WW], BF16, tag=f"ratg{mi}", name=f"ratg{mi}_{b}")
                for mi in range(MI)]
    for nn in range(NN):
        yh_c, yl_c = [], []
        for k in range(KC):
            ysl = cx.y_t[k][:, nn * NT:(nn + 1) * NT]
            h = hlp.tile([P, NT], BF16, tag="hlc", name=f"h_{b}_{nn}_{k}")
            nc.scalar.copy(h[:], ysl)
            l = hlp.tile([P, NT], BF16, tag="hlc", name=f"l_{b}_{nn}_{k}")
            (nc.vector if k % 2 == 0 else nc.gpsimd).tensor_sub(l[:], ysl, h[:])
            yh_c.append(h); yl_c.append(l)
        for mi in range(MI):
            pt = ps_conv.tile([P, NT], F32, tag=f"conv{mi}", name=f"pth{mi}_{b}_{nn}")
            for k in range(KC):
                nc.tensor.matmul(pt[:], wth_t[k][:, mi * P:(mi + 1) * P],
                                 yh_c[k][:], start=(k == 0), stop=False)
            for k in range(KC):
                nc.tensor.matmul(pt[:], wth_t[k][:, mi * P:(mi + 1) * P],
                                 yl_c[k][:], start=False, stop=(k == KC - 1))
            spp_reduce_from_psum(nc, pt, nn, cx.rat_th[mi])
        for mi in range(MI):
            pg = ps_conv.tile([P, NT], F32, tag=f"conv{mi}", name=f"pg{mi}_{b}_{nn}")
            for k in range(KC):
                nc.tensor.matmul(pg[:], wg_t[k][:, mi * P:(mi + 1) * P],
                                 yh_c[k][:], start=(k == 0), stop=(k == KC - 1))
            spp_reduce_from_psum(nc, pg, nn, cx.rat_g[mi])


def phase_binpool_m1(cx, nc, pp, ps, wphi_t, **_):
    b = cx.b
    _, _, ps_small = ps
    cx.spp = []
    cx.g_bf = []
    for mi in range(MI):
        rbt = pp.tile([P, RB_TOT * WW], F32, tag=f"rbth{mi}", name=f"rbth{mi}_{b}")
        sppt = pp.tile([P, S], F32, tag=f"sppth{mi}", name=f"sppth{mi}_{b}")
        spp_bins(nc, cx.rat_th[mi], rbt, sppt)
        cx.spp.append(sppt)
        rbg = pp.tile([P, RB_TOT * WW], BF16, tag=f"rbg{mi}", name=f"rbg{mi}_{b}")
        gbf = pp.tile([P, S], BF16, tag=f"gbf{mi}", name=f"gbf{mi}_{b}")
        spp_bins(nc, cx.rat_g[mi], rbg, gbf)
        cx.g_bf.append(gbf)
    cx.m1_bf = []
    for mc in range(KC):
        pm = ps_small.tile([P, S], F32, tag="psmall", name=f"pm1_{b}_{mc}")
        for k in range(MI):
            nc.tensor.matmul(pm[:], wphi_t[k][:, mc * P:(mc + 1) * P],
                             cx.spp[k][:], start=(k == 0), stop=(k == MI - 1))
        m = pp.tile([P, S], BF16, tag=f"m1_{mc}", name=f"m1b_{b}_{mc}")
        nc.scalar.copy(m[:], pm[:])
        cx.m1_bf.append(m)


def phase_scores(cx, nc, x_ext, iop, hlp, pp, ps, **_):
    b = cx.b
    _, ps_attn, _ = ps
    cx.x_t = []
    for k in range(KC):
        t = iop.tile([P, N], F32, tag="io32", name=f"xt_{b}_{k}")
        nc.sync.dma_start(t[:, 0:N // 2], x_ext[b, k * P:(k + 1) * P, 0:N // 2])
        nc.sync.dma_start(t[:, N // 2:N], x_ext[b, k * P:(k + 1) * P, N // 2:N])
        cx.x_t.append(t)
    cx.sc_sb = pp.tile([S, N], F32, tag="scsb", name=f"scsb_{b}")
    cx.gm = pp.tile([S, 16], F32, tag="gm", name=f"gm_{b}")
    for nn in range(NN):
        xh_c = []
        for k in range(KC):
            xsl = cx.x_t[k][:, nn * NT:(nn + 1) * NT]
            h = hlp.tile([P, NT], BF16, tag="hlc", name=f"xh_{b}_{nn}_{k}")
            nc.scalar.copy(h[:], xsl)
            xh_c.append(h)
        psc = ps_attn.tile([S, NT], F32, tag="pattn", name=f"psc_{b}_{nn}")
        for k in range(KC):
            nc.tensor.matmul(psc[:], cx.m1_bf[k][:], xh_c[k][:],
                             start=(k == 0), stop=(k == KC - 1))
        nc.vector.reduce_max(cx.gm[:, nn:nn + 1], psc[:], axis=AX.X)
        nc.scalar.copy(cx.sc_sb[:, nn * NT:(nn + 1) * NT], psc[:])


def phase_softmax_m2(cx, nc, pp, ps, wmk_t, **_):
    b = cx.b
    _, _, ps_small = ps
    gmax = pp.tile([S, 1], F32, tag="gmax", name=f"gmax_{b}")
    nc.vector.reduce_max(gmax[:], cx.gm[:, 0:NN], axis=AX.X)
    ngmax = pp.tile([S, 1], F32, tag="ngmax", name=f"ngmax_{b}")
    nc.vector.tensor_scalar_mul(ngmax[:], gmax[:], -1.0)
    cx.e_bf = pp.tile([S, N], BF16, tag="ebf", name=f"ebf_{b}")
    dsum = pp.tile([S, 1], F32, tag="dsum", name=f"dsum_{b}")
    nc.scalar.activation(cx.e_bf[:], cx.sc_sb[:], mybir.ActivationFunctionType.Exp,
                         bias=ngmax[:], scale=1.0, accum_out=dsum[:])
    rden = pp.tile([S, 1], F32, tag="rden", name=f"rden_{b}")
    nc.vector.reciprocal(rden[:], dsum[:])
    pm2 = ps_small.tile([S, C], F32, tag="psmall", name=f"pm2_{b}")
    for k in range(MI):
        nc.tensor.matmul(pm2[:], cx.g_bf[k][:], wmk_t[k][:],
                         start=(k == 0), stop=(k == MI - 1))
    cx.m2_bf = pp.tile([S, C], BF16, tag="m2", name=f"m2_{b}")
    nc.vector.tensor_scalar_mul(cx.m2_bf[:], pm2[:], rden[:])


def phase_mask_out(cx, nc, out_ext, osp, ps, **_):
    b = cx.b
    _, ps_attn, _ = ps
    for mc in range(KC):
        for nn in range(NN):
            pk = ps_attn.tile([P, NT], F32, tag="pattn", name=f"pk_{b}_{mc}_{nn}")
            nc.tensor.matmul(pk[:], cx.m2_bf[:, mc * P:(mc + 1) * P],
                             cx.e_bf[:, nn * NT:(nn + 1) * NT],
                             start=True, stop=True)
            xsl = cx.x_t[mc][:, nn * NT:(nn + 1) * NT]
            if (mc + nn) % 2 == 0:
                nc.vector.tensor_add(xsl, pk[:], xsl)
                nc.sync.dma_start(
                    out_ext[b, mc * P:(mc + 1) * P, nn * NT:(nn + 1) * NT], xsl)
            else:
                o = osp.tile([P, NT], F32, tag="ostg", name=f"ost_{b}_{mc}_{nn}")
                nc.scalar.copy(o[:], pk[:])
                nc.gpsimd.tensor_add(o[:], o[:], xsl)
                nc.sync.dma_start(
                    out_ext[b, mc * P:(mc + 1) * P, nn * NT:(nn + 1) * NT], o[:])


_NC_CACHE = {}


def _get_nc():
    if "nc" not in _NC_CACHE:
        _NC_CACHE["nc"] = build()
    return _NC_CACHE["nc"]


def kernel(x, y, w_phi, w_theta, w_g, w_mask):
    x = np.ascontiguousarray(np.asarray(x, dtype=np.float32))
    y = np.ascontiguousarray(np.asarray(y, dtype=np.float32))
    bf = ml_dtypes.bfloat16
    wthT = np.ascontiguousarray(np.asarray(w_theta, np.float32).T).astype(bf)
    wgT = np.ascontiguousarray(np.asarray(w_g, np.float32).T).astype(bf)
    wphi = np.ascontiguousarray(np.asarray(w_phi, np.float32))
    wmkT = np.ascontiguousarray(np.asarray(w_mask, np.float32).T).astype(bf)

    nc = _get_nc()
    in_maps = []
    for c in range(NCORES):
        sl = slice(c * NB, (c + 1) * NB)
        in_maps.append({
            "x": x[sl].reshape(NB, C, N),
            "y": y[sl].reshape(NB, C, N),
            "wthT": wthT, "wgT": wgT, "wphi": wphi, "wmkT": wmkT,
        })
    res = run_bass_kernel_spmd(nc, in_maps, core_ids=list(range(NCORES)))
    out = np.concatenate([r["out"].reshape(NB, C, HH, WW) for r in res.results],
                         axis=0)
    return out


# revision 9
# speedup vs baseline: 1.4056x; 1.0065x over previous
"""AFNB (asymmetric fusion non-local block) Trainium2 kernel, 8-core SPMD.

Data-parallel over batch: 16 batches -> 2 per core, no collectives.

Algebra (per batch, softmax over the QUERY axis allows folding):
  theta = w_theta @ y        [IC, N]   (split2: bf16 weights, y = yh+yl bf16)
  th_spp = SPP(theta)        [IC, S]   (exact fp32 max-pool)
  g    = w_g @ y             [IC, N]   (bf16)
  g_spp = SPP(g)             [IC, S]   (bf16)
  M1T  = w_phi^T @ th_spp    [C, S]    (fp32)
  scoresT = M1T^T @ x        [S, N]    (split2: bf16 M1, x = xh+xl)
  e    = exp(scoresT - rowmax)         (ACT, accum -> denom)
  M2T  = (g_spp^T @ w_mask^T) / denom  [S, C] (bf16)
  out  = M2T^T @ e + x       [C, N]
"""

import numpy as np
import ml_dtypes

import concourse.bass as bass
import concourse.tile as tile
from concourse import bacc, mybir
from concourse.ap import AP
from concourse.bass_utils import run_bass_kernel_spmd

F32 = mybir.dt.float32
BF16 = mybir.dt.bfloat16
AX = mybir.AxisListType
OP = mybir.AluOpType

B, C, HH, WW = 16, 512, 64, 64
N = HH * WW
IC = 256
NCORES = 8
NB = B // NCORES  # batches per core
P = 128
KC = C // P   # 4 contraction chunks over C
MI = IC // P  # 2 chunks over IC
NN = 8        # n-chunks
NT = N // NN  # 512
OUT_SIZES = [1, 3, 6, 8]
S = sum(o * o for o in OUT_SIZES)  # 110
SPP_OFF = {1: 0, 3: 1, 6: 10, 8: 46}


def _bounds(n, o):
    return [((i * n) // o, ((i + 1) * n + o - 1) // o) for i in range(o)]


def _atoms():
    bs = set()
    for o in OUT_SIZES:
        for s, e in _bounds(HH, o):
            bs.add(s); bs.add(e)
    bs = sorted(bs)
    return [(bs[i], bs[i + 1]) for i in range(len(bs) - 1)]


ATOMS = _atoms()          # 16 atomic row intervals
NA = len(ATOMS)


def _bin_atom_ranges(o):
    """For each bin of size-o pooling: (first_atom_idx, last_atom_idx_excl)."""
    out = []
    for s, e in _bounds(HH, o):
        a0 = next(i for i, (as_, _) in enumerate(ATOMS) if as_ == s)
        a1 = next(i for i, (_, ae) in enumerate(ATOMS) if ae == e) + 1
        out.append((a0, a1))
    return out


def _grouped(items):
    """Group indices j of (start, length) items into classes {j = r mod m} where
    each class has constant length and arithmetic starts. Returns list of
    (j0, m, cnt, start0, dstart, length)."""
    n = len(items)
    for m in range(1, n + 1):
        groups = []
        ok = True
        for r in range(m):
            js = list(range(r, n, m))
            lens = {items[j][1] for j in js}
            if len(lens) != 1:
                ok = False; break
            starts = [items[j][0] for j in js]
            d = starts[1] - starts[0] if len(starts) > 1 else 0
            if any(starts[i + 1] - starts[i] != d for i in range(len(starts) - 1)):
                ok = False; break
            groups.append((r, m, len(js), starts[0], d, lens.pop()))
        if ok:
            return groups
    raise AssertionError


ROW_GROUPS = {o: _grouped([(a0, a1 - a0) for a0, a1 in _bin_atom_ranges(o)])
              for o in OUT_SIZES}
COL_GROUPS = {o: _grouped([(s, e - s) for s, e in _bounds(WW, o)])
              for o in OUT_SIZES}
RB_OFF = {}  # row-bin output offset (units of 64 cols) per o
_off = 0
for _o in OUT_SIZES:
    RB_OFF[_o] = _off
    _off += _o
RB_TOT = _off  # 18


def mk(ap_base, off_elems, dims):
    """Custom free-dim AP over a tile's base AP: dims = [(step, count), ...]."""
    part = list(ap_base.ap[0])
    return AP(tensor=ap_base.tensor, offset=ap_base.offset + off_elems,
              ap=[part] + [[s, c] for s, c in dims])


def build():
    nc = bacc.Bacc("TRN2", target_bir_lowering=False, debug=False,
                   num_devices=NCORES)
    x_ext = nc.declare_dram_parameter("x", [NB, C, N], F32, isOutput=False)
    y_ext = nc.declare_dram_parameter("y", [NB, C, N], F32, isOutput=False)
    wth_ext = nc.declare_dram_parameter("wthT", [C, IC], BF16, isOutput=False)
    wg_ext = nc.declare_dram_parameter("wgT", [C, IC], BF16, isOutput=False)
    wphi_ext = nc.declare_dram_parameter("wphi", [IC, C], F32, isOutput=False)
    wmk_ext = nc.declare_dram_parameter("wmkT", [IC, C], BF16, isOutput=False)
    out_ext = nc.declare_dram_parameter("out", [NB, C, N], F32, isOutput=True)

    with tile.TileContext(nc) as tc:
        with (
            tc.tile_pool(name="w", bufs=1) as wp,
            tc.tile_pool(name="io32", bufs=6) as iop,
            tc.tile_pool(name="hlc", bufs=10) as hlp,
            tc.tile_pool(name="pool", bufs=1) as pp,
            tc.tile_pool(name="attn", bufs=2) as ap_,
            tc.tile_pool(name="ostg", bufs=6) as osp,
            tc.tile_pool(name="psc", bufs=2, space="PSUM") as ps_conv,
            tc.tile_pool(name="psa", bufs=3, space="PSUM") as ps_attn,
            tc.tile_pool(name="pss", bufs=1, space="PSUM") as ps_small,
        ):
            # ---- weights (resident) ----
            wth_t = [wp.tile([P, IC], BF16, tag=f"wth{k}", name=f"wth{k}") for k in range(KC)]
            wg_t = [wp.tile([P, IC], BF16, tag=f"wg{k}", name=f"wg{k}") for k in range(KC)]
            wphi_t = [wp.tile([P, C], F32, tag=f"wphi{k}", name=f"wphi{k}") for k in range(MI)]
            wmk_t = [wp.tile([P, C], BF16, tag=f"wmk{k}", name=f"wmk{k}") for k in range(MI)]
            for k in range(KC):
                nc.sync.dma_start(wth_t[k][:], wth_ext[k * P:(k + 1) * P, :])
                nc.sync.dma_start(wg_t[k][:], wg_ext[k * P:(k + 1) * P, :])
            for k in range(MI):
                nc.sync.dma_start(wphi_t[k][:], wphi_ext[k * P:(k + 1) * P, :])
                nc.sync.dma_start(wmk_t[k][:], wmk_ext[k * P:(k + 1) * P, :])

            ps = (ps_conv, ps_attn, ps_small)
            ctxs = [BatchCtx(b) for b in range(NB)]
            A = dict(nc=nc, x_ext=x_ext, y_ext=y_ext, out_ext=out_ext,
                     wth_t=wth_t, wg_t=wg_t, wphi_t=wphi_t, wmk_t=wmk_t,
                     iop=iop, hlp=hlp, pp=pp, osp=osp, ps=ps)
            phase_conv(ctxs[0], **A)
            phase_binpool_m1(ctxs[0], **A)
            phase_scores(ctxs[0], **A)
            phase_softmax_m2(ctxs[0], **A)
            phase_conv(ctxs[1], **A)
            phase_binpool_m1(ctxs[1], **A)
            phase_mask_out(ctxs[0], **A)
            phase_scores(ctxs[1], **A)
            phase_softmax_m2(ctxs[1], **A)
            phase_mask_out(ctxs[1], **A)

    nc.compile()
    return nc


class BatchCtx:
    def __init__(self, b):
        self.b = b


def spp_reduce_from_psum(nc, pt, nn, ratom):
    """Stage R: row-atom max-pool from a conv psum chunk [128,512] (8 image
    rows). ratom layout is atom-INNER: ratom[p, w*NA + a], so B1's reduce
    (over atoms) streams contiguous runs."""
    base = pt[:]
    for ai, (s, e) in enumerate(ATOMS):
        if s >= 8 * nn and e <= 8 * (nn + 1):
            ls = s - 8 * nn
            src = mk(base, ls * WW, [(1, WW), (WW, e - s)])
            dst = mk(ratom[:], ai, [(NA, WW)])
            nc.vector.reduce_max(dst, src, axis=AX.X)


def spp_bins(nc, ratom, rb, spp):
    """B1 (row bins from atoms; atom axis innermost+contiguous) then B2
    (col bins over rb[p, w*RB_TOT + bin], strided uniform groups)."""
    for o in OUT_SIZES:
        for (r, m, cnt, a0, da, ln) in ROW_GROUPS[o]:
            src = mk(ratom[:], a0, [(da, cnt), (NA, WW), (1, ln)])
            dst = mk(rb[:], RB_OFF[o] + r, [(m, cnt), (RB_TOT, WW)])
            nc.vector.reduce_max(dst, src, axis=AX.X)
        for (r, m, cnt, s0, ds, ln) in COL_GROUPS[o]:
            src = mk(rb[:], RB_OFF[o] + r + s0 * RB_TOT,
                     [(1, o), (ds * RB_TOT, cnt), (RB_TOT, ln)])
            dst = mk(spp[:], SPP_OFF[o] + r, [(o, o), (m, cnt)])
            nc.vector.reduce_max(dst, src, axis=AX.X)


def phase_conv(cx, nc, y_ext, iop, hlp, pp, ps, wth_t, wg_t, **_):
    b = cx.b
    ps_conv, _, _ = ps
    cx.y_t = []
    for k in range(KC):
        t = iop.tile([P, N], F32, tag="io32", name=f"yt_{b}_{k}")
        for q in range(4):
            nc.sync.dma_start(t[:, q * N // 4:(q + 1) * N // 4],
                              y_ext[b, k * P:(k + 1) * P, q * N // 4:(q + 1) * N // 4])
        cx.y_t.append(t)
    cx.rat_th = [pp.tile([P, NA * WW], F32, tag=f"rath{mi}", name=f"rath{mi}_{b}")
                 for mi in range(MI)]
    cx.rat_g = [pp.tile([P, NA * WW], BF16, tag=f"ratg{mi}", name=f"ratg{mi}_{b}")
                for mi in range(MI)]
    for np_ in range(NN // 2):
        nns = (2 * np_, 2 * np_ + 1)
        yh_c = {}; yl_c = {}
        for nn in nns:
            for k in range(KC):
                ysl = cx.y_t[k][:, nn * NT:(nn + 1) * NT]
                h = hlp.tile([P, NT], BF16, tag="hlc", name=f"h_{b}_{nn}_{k}")
                nc.scalar.copy(h[:], ysl)
                l = hlp.tile([P, NT], BF16, tag="hlc", name=f"l_{b}_{nn}_{k}")
                (nc.vector if k % 2 == 0 else nc.gpsimd).tensor_sub(l[:], ysl, h[:])
                yh_c[nn, k] = h; yl_c[nn, k] = l
        for mi in range(MI):
            pt = {nn: ps_conv.tile([P, NT], F32, tag=f"conv{mi}",
                                   name=f"pth{mi}_{b}_{nn}") for nn in nns}
            for k in range(KC):
                for nn in nns:
                    nc.tensor.matmul(pt[nn][:], wth_t[k][:, mi * P:(mi + 1) * P],
                                     yh_c[nn, k][:], start=(k == 0), stop=False)
            for k in range(KC):
                for nn in nns:
                    nc.tensor.matmul(pt[nn][:], wth_t[k][:, mi * P:(mi + 1) * P],
                                     yl_c[nn, k][:], start=False, stop=(k == KC - 1))
            for nn in nns:
                spp_reduce_from_psum(nc, pt[nn], nn, cx.rat_th[mi])
        for mi in range(MI):
            pg = {nn: ps_conv.tile([P, NT], F32, tag=f"conv{mi}",
                                   name=f"pg{mi}_{b}_{nn}") for nn in nns}
            for k in range(KC):
                for nn in nns:
                    nc.tensor.matmul(pg[nn][:], wg_t[k][:, mi * P:(mi + 1) * P],
                                     yh_c[nn, k][:], start=(k == 0), stop=(k == KC - 1))
            for nn in nns:
                spp_reduce_from_psum(nc, pg[nn], nn, cx.rat_g[mi])


def phase_binpool_m1(cx, nc, pp, ps, wphi_t, **_):
    b = cx.b
    _, _, ps_small = ps
    cx.spp = []
    cx.g_bf = []
    for mi in range(MI):
        rbt = pp.tile([P, RB_TOT * WW], F32, tag=f"rbth{mi}", name=f"rbth{mi}_{b}")
        sppt = pp.tile([P, S], F32, tag=f"sppth{mi}", name=f"sppth{mi}_{b}")
        spp_bins(nc, cx.rat_th[mi], rbt, sppt)
        cx.spp.append(sppt)
        rbg = pp.tile([P, RB_TOT * WW], BF16, tag=f"rbg{mi}", name=f"rbg{mi}_{b}")
        gbf = pp.tile([P, S], BF16, tag=f"gbf{mi}", name=f"gbf{mi}_{b}")
        spp_bins(nc, cx.rat_g[mi], rbg, gbf)
        cx.g_bf.append(gbf)
    cx.m1_bf = []
    for mc in range(KC):
        pm = ps_small.tile([P, S], F32, tag="psmall", name=f"pm1_{b}_{mc}")
        for k in range(MI):
            nc.tensor.matmul(pm[:], wphi_t[k][:, mc * P:(mc + 1) * P],
                             cx.spp[k][:], start=(k == 0), stop=(k == MI - 1))
        m = pp.tile([P, S], BF16, tag=f"m1_{mc}", name=f"m1b_{b}_{mc}")
        nc.scalar.copy(m[:], pm[:])
        cx.m1_bf.append(m)


def phase_scores(cx, nc, x_ext, iop, hlp, pp, ps, **_):
    b = cx.b
    _, ps_attn, _ = ps
    cx.x_t = []
    for k in range(KC):
        t = iop.tile([P, N], F32, tag="io32", name=f"xt_{b}_{k}")
        nc.sync.dma_start(t[:, 0:N // 2], x_ext[b, k * P:(k + 1) * P, 0:N // 2])
        nc.sync.dma_start(t[:, N // 2:N], x_ext[b, k * P:(k + 1) * P, N // 2:N])
        cx.x_t.append(t)
    cx.sc_sb = pp.tile([S, N], F32, tag="scsb", name=f"scsb_{b}")
    cx.gm = pp.tile([S, 16], F32, tag="gm", name=f"gm_{b}")
    for nn in range(NN):
        xh_c = []
        for k in range(KC):
            xsl = cx.x_t[k][:, nn * NT:(nn + 1) * NT]
            h = hlp.tile([P, NT], BF16, tag="hlc", name=f"xh_{b}_{nn}_{k}")
            nc.scalar.copy(h[:], xsl)
            xh_c.append(h)
        psc = ps_attn.tile([S, NT], F32, tag="pattn", name=f"psc_{b}_{nn}")
        for k in range(KC):
            nc.tensor.matmul(psc[:], cx.m1_bf[k][:], xh_c[k][:],
                             start=(k == 0), stop=(k == KC - 1))
        nc.vector.reduce_max(cx.gm[:, nn:nn + 1], psc[:], axis=AX.X)
        nc.scalar.copy(cx.sc_sb[:, nn * NT:(nn + 1) * NT], psc[:])


def phase_softmax_m2(cx, nc, pp, ps, wmk_t, **_):
    b = cx.b
    _, _, ps_small = ps
    gmax = pp.tile([S, 1], F32, tag="gmax", name=f"gmax_{b}")
    nc.vector.reduce_max(gmax[:], cx.gm[:, 0:NN], axis=AX.X)
    ngmax = pp.tile([S, 1], F32, tag="ngmax", name=f"ngmax_{b}")
    nc.vector.tensor_scalar_mul(ngmax[:], gmax[:], -1.0)
    cx.e_bf = pp.tile([S, N], BF16, tag="ebf", name=f"ebf_{b}")
    dsum = pp.tile([S, 1], F32, tag="dsum", name=f"dsum_{b}")
    nc.scalar.activation(cx.e_bf[:], cx.sc_sb[:], mybir.ActivationFunctionType.Exp,
                         bias=ngmax[:], scale=1.0, accum_out=dsum[:])
    rden = pp.tile([S, 1], F32, tag="rden", name=f"rden_{b}")
    nc.vector.reciprocal(rden[:], dsum[:])
    pm2 = ps_small.tile([S, C], F32, tag="psmall", name=f"pm2_{b}")
    for k in range(MI):
        nc.tensor.matmul(pm2[:], cx.g_bf[k][:], wmk_t[k][:],
                         start=(k == 0), stop=(k == MI - 1))
    cx.m2_bf = pp.tile([S, C], BF16, tag="m2", name=f"m2_{b}")
    nc.vector.tensor_scalar_mul(cx.m2_bf[:], pm2[:], rden[:])


def phase_mask_out(cx, nc, out_ext, osp, ps, **_):
    b = cx.b
    _, ps_attn, _ = ps
    for mc in range(KC):
        for nn in range(NN):
            pk = ps_attn.tile([P, NT], F32, tag="pattn", name=f"pk_{b}_{mc}_{nn}")
            nc.tensor.matmul(pk[:], cx.m2_bf[:, mc * P:(mc + 1) * P],
                             cx.e_bf[:, nn * NT:(nn + 1) * NT],
                             start=True, stop=True)
            xsl = cx.x_t[mc][:, nn * NT:(nn + 1) * NT]
            if (mc + nn) % 2 == 0:
                nc.vector.tensor_add(xsl, pk[:], xsl)
                nc.sync.dma_start(
                    out_ext[b, mc * P:(mc + 1) * P, nn * NT:(nn + 1) * NT], xsl)
            else:
                o = osp.tile([P, NT], F32, tag="ostg", name=f"ost_{b}_{mc}_{nn}")
                nc.scalar.copy(o[:], pk[:])
                nc.gpsimd.tensor_add(o[:], o[:], xsl)
                nc.sync.dma_start(
                    out_ext[b, mc * P:(mc + 1) * P, nn * NT:(nn + 1) * NT], o[:])


_NC_CACHE = {}


def _get_nc():
    if "nc" not in _NC_CACHE:
        _NC_CACHE["nc"] = build()
    return _NC_CACHE["nc"]


def kernel(x, y, w_phi, w_theta, w_g, w_mask):
    x = np.ascontiguousarray(np.asarray(x, dtype=np.float32))
    y = np.ascontiguousarray(np.asarray(y, dtype=np.float32))
    bf = ml_dtypes.bfloat16
    wthT = np.ascontiguousarray(np.asarray(w_theta, np.float32).T).astype(bf)
    wgT = np.ascontiguousarray(np.asarray(w_g, np.float32).T).astype(bf)
    wphi = np.ascontiguousarray(np.asarray(w_phi, np.float32))
    wmkT = np.ascontiguousarray(np.asarray(w_mask, np.float32).T).astype(bf)

    nc = _get_nc()
    in_maps = []
    for c in range(NCORES):
        sl = slice(c * NB, (c + 1) * NB)
        in_maps.append({
            "x": x[sl].reshape(NB, C, N),
            "y": y[sl].reshape(NB, C, N),
            "wthT": wthT, "wgT": wgT, "wphi": wphi, "wmkT": wmkT,
        })
    res = run_bass_kernel_spmd(nc, in_maps, core_ids=list(range(NCORES)))
    out = np.concatenate([r["out"].reshape(NB, C, HH, WW) for r in res.results],
                         axis=0)
    return out


# revision 10
# speedup vs baseline: 1.4259x; 1.0145x over previous
"""AFNB (asymmetric fusion non-local block) Trainium2 kernel, 8-core SPMD.

Data-parallel over batch: 16 batches -> 2 per core, no collectives.

Algebra (per batch, softmax over the QUERY axis allows folding):
  theta = w_theta @ y        [IC, N]   (split2: bf16 weights, y = yh+yl bf16)
  th_spp = SPP(theta)        [IC, S]   (exact fp32 max-pool)
  g    = w_g @ y             [IC, N]   (bf16)
  g_spp = SPP(g)             [IC, S]   (bf16)
  M1T  = w_phi^T @ th_spp    [C, S]    (fp32)
  scoresT = M1T^T @ x        [S, N]    (split2: bf16 M1, x = xh+xl)
  e    = exp(scoresT - rowmax)         (ACT, accum -> denom)
  M2T  = (g_spp^T @ w_mask^T) / denom  [S, C] (bf16)
  out  = M2T^T @ e + x       [C, N]
"""

import numpy as np
import ml_dtypes

import concourse.bass as bass
import concourse.tile as tile
from concourse import bacc, mybir
from concourse.ap import AP
from concourse.bass_utils import run_bass_kernel_spmd

F32 = mybir.dt.float32
BF16 = mybir.dt.bfloat16
AX = mybir.AxisListType
OP = mybir.AluOpType

B, C, HH, WW = 16, 512, 64, 64
N = HH * WW
IC = 256
NCORES = 8
NB = B // NCORES  # batches per core
P = 128
KC = C // P   # 4 contraction chunks over C
MI = IC // P  # 2 chunks over IC
NN = 8        # n-chunks
NT = N // NN  # 512
OUT_SIZES = [1, 3, 6, 8]
S = sum(o * o for o in OUT_SIZES)  # 110
SPP_OFF = {1: 0, 3: 1, 6: 10, 8: 46}


def _bounds(n, o):
    return [((i * n) // o, ((i + 1) * n + o - 1) // o) for i in range(o)]


def _atoms():
    bs = set()
    for o in OUT_SIZES:
        for s, e in _bounds(HH, o):
            bs.add(s); bs.add(e)
    bs = sorted(bs)
    return [(bs[i], bs[i + 1]) for i in range(len(bs) - 1)]


ATOMS = _atoms()          # 16 atomic row intervals
NA = len(ATOMS)


def _bin_atom_ranges(o):
    """For each bin of size-o pooling: (first_atom_idx, last_atom_idx_excl)."""
    out = []
    for s, e in _bounds(HH, o):
        a0 = next(i for i, (as_, _) in enumerate(ATOMS) if as_ == s)
        a1 = next(i for i, (_, ae) in enumerate(ATOMS) if ae == e) + 1
        out.append((a0, a1))
    return out


def _grouped(items):
    """Group indices j of (start, length) items into classes {j = r mod m} where
    each class has constant length and arithmetic starts. Returns list of
    (j0, m, cnt, start0, dstart, length)."""
    n = len(items)
    for m in range(1, n + 1):
        groups = []
        ok = True
        for r in range(m):
            js = list(range(r, n, m))
            lens = {items[j][1] for j in js}
            if len(lens) != 1:
                ok = False; break
            starts = [items[j][0] for j in js]
            d = starts[1] - starts[0] if len(starts) > 1 else 0
            if any(starts[i + 1] - starts[i] != d for i in range(len(starts) - 1)):
                ok = False; break
            groups.append((r, m, len(js), starts[0], d, lens.pop()))
        if ok:
            return groups
    raise AssertionError


ROW_GROUPS = {o: _grouped([(a0, a1 - a0) for a0, a1 in _bin_atom_ranges(o)])
              for o in OUT_SIZES}
COL_GROUPS = {o: _grouped([(s, e - s) for s, e in _bounds(WW, o)])
              for o in OUT_SIZES}
RB_OFF = {}  # row-bin output offset (units of 64 cols) per o
_off = 0
for _o in OUT_SIZES:
    RB_OFF[_o] = _off
    _off += _o
RB_TOT = _off  # 18


def mk(ap_base, off_elems, dims):
    """Custom free-dim AP over a tile's base AP: dims = [(step, count), ...]."""
    part = list(ap_base.ap[0])
    return AP(tensor=ap_base.tensor, offset=ap_base.offset + off_elems,
              ap=[part] + [[s, c] for s, c in dims])


def build():
    nc = bacc.Bacc("TRN2", target_bir_lowering=False, debug=False,
                   num_devices=NCORES)
    x_ext = nc.declare_dram_parameter("x", [NB, C, N], F32, isOutput=False)
    y_ext = nc.declare_dram_parameter("y", [NB, C, N], F32, isOutput=False)
    wth_ext = nc.declare_dram_parameter("wthT", [C, IC], BF16, isOutput=False)
    wg_ext = nc.declare_dram_parameter("wgT", [C, IC], BF16, isOutput=False)
    wphi_ext = nc.declare_dram_parameter("wphi", [IC, C], F32, isOutput=False)
    wmk_ext = nc.declare_dram_parameter("wmkT", [IC, C], BF16, isOutput=False)
    out_ext = nc.declare_dram_parameter("out", [NB, C, N], F32, isOutput=True)

    with tile.TileContext(nc) as tc:
        with (
            tc.tile_pool(name="w", bufs=1) as wp,
            tc.tile_pool(name="io32", bufs=6) as iop,
            tc.tile_pool(name="hlc", bufs=10) as hlp,
            tc.tile_pool(name="pool", bufs=1) as pp,
            tc.tile_pool(name="attn", bufs=2) as ap_,
            tc.tile_pool(name="ostg", bufs=6) as osp,
            tc.tile_pool(name="psc", bufs=2, space="PSUM") as ps_conv,
            tc.tile_pool(name="psa", bufs=3, space="PSUM") as ps_attn,
            tc.tile_pool(name="pss", bufs=1, space="PSUM") as ps_small,
        ):
            # ---- weights (resident) ----
            wth_t = [wp.tile([P, IC], BF16, tag=f"wth{k}", name=f"wth{k}") for k in range(KC)]
            wg_t = [wp.tile([P, IC], BF16, tag=f"wg{k}", name=f"wg{k}") for k in range(KC)]
            wphi_t = [wp.tile([P, C], F32, tag=f"wphi{k}", name=f"wphi{k}") for k in range(MI)]
            wmk_t = [wp.tile([P, C], BF16, tag=f"wmk{k}", name=f"wmk{k}") for k in range(MI)]
            for k in range(KC):
                nc.sync.dma_start(wth_t[k][:], wth_ext[k * P:(k + 1) * P, :])
                nc.sync.dma_start(wg_t[k][:], wg_ext[k * P:(k + 1) * P, :])
            for k in range(MI):
                nc.sync.dma_start(wphi_t[k][:], wphi_ext[k * P:(k + 1) * P, :])
                nc.sync.dma_start(wmk_t[k][:], wmk_ext[k * P:(k + 1) * P, :])

            ps = (ps_conv, ps_attn, ps_small)
            ctxs = [BatchCtx(b) for b in range(NB)]
            A = dict(nc=nc, x_ext=x_ext, y_ext=y_ext, out_ext=out_ext,
                     wth_t=wth_t, wg_t=wg_t, wphi_t=wphi_t, wmk_t=wmk_t,
                     iop=iop, hlp=hlp, pp=pp, osp=osp, ps=ps)
            phase_conv(ctxs[0], **A)
            phase_binpool_m1(ctxs[0], **A)
            phase_scores(ctxs[0], **A)
            phase_softmax_m2(ctxs[0], **A)
            phase_conv(ctxs[1], **A)
            phase_binpool_m1(ctxs[1], **A)
            phase_mask_out(ctxs[0], **A)
            phase_scores(ctxs[1], **A)
            phase_softmax_m2(ctxs[1], **A)
            phase_mask_out(ctxs[1], **A)

    nc.compile()
    return nc


class BatchCtx:
    def __init__(self, b):
        self.b = b


def spp_reduce_from_psum(nc, pt, nn, ratom):
    """Stage R: row-atom max-pool from a conv psum chunk [128,512] (8 image
    rows). ratom layout is atom-INNER: ratom[p, w*NA + a], so B1's reduce
    (over atoms) streams contiguous runs."""
    base = pt[:]
    for ai, (s, e) in enumerate(ATOMS):
        if s >= 8 * nn and e <= 8 * (nn + 1):
            ls = s - 8 * nn
            src = mk(base, ls * WW, [(1, WW), (WW, e - s)])
            dst = mk(ratom[:], ai, [(NA, WW)])
            nc.vector.reduce_max(dst, src, axis=AX.X)


def spp_bins(nc, ratom, rb, spp):
    """B1 (row bins from atoms; atom axis innermost+contiguous) then B2
    (col bins over rb[p, w*RB_TOT + bin], strided uniform groups)."""
    for o in OUT_SIZES:
        for (r, m, cnt, a0, da, ln) in ROW_GROUPS[o]:
            src = mk(ratom[:], a0, [(da, cnt), (NA, WW), (1, ln)])
            dst = mk(rb[:], RB_OFF[o] + r, [(m, cnt), (RB_TOT, WW)])
            nc.vector.reduce_max(dst, src, axis=AX.X)
        for (r, m, cnt, s0, ds, ln) in COL_GROUPS[o]:
            src = mk(rb[:], RB_OFF[o] + r + s0 * RB_TOT,
                     [(1, o), (ds * RB_TOT, cnt), (RB_TOT, ln)])
            dst = mk(spp[:], SPP_OFF[o] + r, [(o, o), (m, cnt)])
            nc.vector.reduce_max(dst, src, axis=AX.X)


def phase_conv(cx, nc, y_ext, iop, hlp, pp, ps, wth_t, wg_t, **_):
    b = cx.b
    ps_conv, _, _ = ps
    cx.y_t = []
    for k in range(KC):
        t = iop.tile([P, N], F32, tag="io32", name=f"yt_{b}_{k}")
        for q in range(4):
            nc.sync.dma_start(t[:, q * N // 4:(q + 1) * N // 4],
                              y_ext[b, k * P:(k + 1) * P, q * N // 4:(q + 1) * N // 4])
        cx.y_t.append(t)
    cx.rat_th = [pp.tile([P, NA * WW], F32, tag=f"rath{mi}", name=f"rath{mi}_{b}")
                 for mi in range(MI)]
    cx.rat_g = [pp.tile([P, NA * WW], BF16, tag=f"ratg{mi}", name=f"ratg{mi}_{b}")
                for mi in range(MI)]
    for np_ in range(NN // 2):
        nns = (2 * np_, 2 * np_ + 1)
        yh_c = {}; yl_c = {}
        for nn in nns:
            for k in range(KC):
                ysl = cx.y_t[k][:, nn * NT:(nn + 1) * NT]
                h = hlp.tile([P, NT], BF16, tag="hlc", name=f"h_{b}_{nn}_{k}")
                nc.scalar.copy(h[:], ysl)
                l = hlp.tile([P, NT], BF16, tag="hlc", name=f"l_{b}_{nn}_{k}")
                nc.gpsimd.tensor_sub(l[:], ysl, h[:])
                yh_c[nn, k] = h; yl_c[nn, k] = l
        for mi in range(MI):
            pt = {nn: ps_conv.tile([P, NT], F32, tag=f"conv{mi}",
                                   name=f"pth{mi}_{b}_{nn}") for nn in nns}
            for k in range(KC):
                for nn in nns:
                    nc.tensor.matmul(pt[nn][:], wth_t[k][:, mi * P:(mi + 1) * P],
                                     yh_c[nn, k][:], start=(k == 0), stop=False)
            for k in range(KC):
                for nn in nns:
                    nc.tensor.matmul(pt[nn][:], wth_t[k][:, mi * P:(mi + 1) * P],
                                     yl_c[nn, k][:], start=False, stop=(k == KC - 1))
            for nn in nns:
                spp_reduce_from_psum(nc, pt[nn], nn, cx.rat_th[mi])
        for mi in range(MI):
            pg = {nn: ps_conv.tile([P, NT], F32, tag=f"conv{mi}",
                                   name=f"pg{mi}_{b}_{nn}") for nn in nns}
            for k in range(KC):
                for nn in nns:
                    nc.tensor.matmul(pg[nn][:], wg_t[k][:, mi * P:(mi + 1) * P],
                                     yh_c[nn, k][:], start=(k == 0), stop=(k == KC - 1))
            for nn in nns:
                spp_reduce_from_psum(nc, pg[nn], nn, cx.rat_g[mi])


def phase_binpool_m1(cx, nc, pp, ps, wphi_t, **_):
    b = cx.b
    _, _, ps_small = ps
    cx.spp = []
    cx.g_bf = []
    for mi in range(MI):
        rbt = pp.tile([P, RB_TOT * WW], F32, tag=f"rbth{mi}", name=f"rbth{mi}_{b}")
        sppt = pp.tile([P, S], F32, tag=f"sppth{mi}", name=f"sppth{mi}_{b}")
        spp_bins(nc, cx.rat_th[mi], rbt, sppt)
        cx.spp.append(sppt)
        rbg = pp.tile([P, RB_TOT * WW], BF16, tag=f"rbg{mi}", name=f"rbg{mi}_{b}")
        gbf = pp.tile([P, S], BF16, tag=f"gbf{mi}", name=f"gbf{mi}_{b}")
        spp_bins(nc, cx.rat_g[mi], rbg, gbf)
        cx.g_bf.append(gbf)
    cx.m1_bf = []
    for mc in range(KC):
        pm = ps_small.tile([P, S], F32, tag="psmall", name=f"pm1_{b}_{mc}")
        for k in range(MI):
            nc.tensor.matmul(pm[:], wphi_t[k][:, mc * P:(mc + 1) * P],
                             cx.spp[k][:], start=(k == 0), stop=(k == MI - 1))
        m = pp.tile([P, S], BF16, tag=f"m1_{mc}", name=f"m1b_{b}_{mc}")
        nc.scalar.copy(m[:], pm[:])
        cx.m1_bf.append(m)


def phase_scores(cx, nc, x_ext, iop, hlp, pp, ps, **_):
    b = cx.b
    _, ps_attn, _ = ps
    cx.x_t = []
    for k in range(KC):
        t = iop.tile([P, N], F32, tag="io32", name=f"xt_{b}_{k}")
        nc.sync.dma_start(t[:, 0:N // 2], x_ext[b, k * P:(k + 1) * P, 0:N // 2])
        nc.sync.dma_start(t[:, N // 2:N], x_ext[b, k * P:(k + 1) * P, N // 2:N])
        cx.x_t.append(t)
    cx.sc_sb = pp.tile([S, N], F32, tag="scsb", name=f"scsb_{b}")
    cx.gm = pp.tile([S, 16], F32, tag="gm", name=f"gm_{b}")
    for nn in range(NN):
        xh_c = []
        for k in range(KC):
            xsl = cx.x_t[k][:, nn * NT:(nn + 1) * NT]
            h = hlp.tile([P, NT], BF16, tag="hlc", name=f"xh_{b}_{nn}_{k}")
            if k % 2 == 0:
                nc.scalar.copy(h[:], xsl)
            else:
                nc.vector.tensor_copy(h[:], xsl)
            xh_c.append(h)
        psc = ps_attn.tile([S, NT], F32, tag="pattn", name=f"psc_{b}_{nn}")
        for k in range(KC):
            nc.tensor.matmul(psc[:], cx.m1_bf[k][:], xh_c[k][:],
                             start=(k == 0), stop=(k == KC - 1))
        nc.vector.reduce_max(cx.gm[:, nn:nn + 1], psc[:], axis=AX.X)
        nc.scalar.copy(cx.sc_sb[:, nn * NT:(nn + 1) * NT], psc[:])


def phase_softmax_m2(cx, nc, pp, ps, wmk_t, **_):
    b = cx.b
    _, _, ps_small = ps
    gmax = pp.tile([S, 1], F32, tag="gmax", name=f"gmax_{b}")
    nc.vector.reduce_max(gmax[:], cx.gm[:, 0:NN], axis=AX.X)
    ngmax = pp.tile([S, 1], F32, tag="ngmax", name=f"ngmax_{b}")
    nc.vector.tensor_scalar_mul(ngmax[:], gmax[:], -1.0)
    cx.e_bf = pp.tile([S, N], BF16, tag="ebf", name=f"ebf_{b}")
    dsum = pp.tile([S, 1], F32, tag="dsum", name=f"dsum_{b}")
    nc.scalar.activation(cx.e_bf[:], cx.sc_sb[:], mybir.ActivationFunctionType.Exp,
                         bias=ngmax[:], scale=1.0, accum_out=dsum[:])
    rden = pp.tile([S, 1], F32, tag="rden", name=f"rden_{b}")
    nc.vector.reciprocal(rden[:], dsum[:])
    pm2 = ps_small.tile([S, C], F32, tag="psmall", name=f"pm2_{b}")
    for k in range(MI):
        nc.tensor.matmul(pm2[:], cx.g_bf[k][:], wmk_t[k][:],
                         start=(k == 0), stop=(k == MI - 1))
    cx.m2_bf = pp.tile([S, C], BF16, tag="m2", name=f"m2_{b}")
    nc.vector.tensor_scalar_mul(cx.m2_bf[:], pm2[:], rden[:])


def phase_mask_out(cx, nc, out_ext, osp, ps, **_):
    b = cx.b
    _, ps_attn, _ = ps
    for mc in range(KC):
        for nn in range(NN):
            pk = ps_attn.tile([P, NT], F32, tag="pattn", name=f"pk_{b}_{mc}_{nn}")
            nc.tensor.matmul(pk[:], cx.m2_bf[:, mc * P:(mc + 1) * P],
                             cx.e_bf[:, nn * NT:(nn + 1) * NT],
                             start=True, stop=True)
            xsl = cx.x_t[mc][:, nn * NT:(nn + 1) * NT]
            if (mc + nn) % 2 == 0:
                nc.vector.tensor_add(xsl, pk[:], xsl)
                nc.sync.dma_start(
                    out_ext[b, mc * P:(mc + 1) * P, nn * NT:(nn + 1) * NT], xsl)
            else:
                o = osp.tile([P, NT], F32, tag="ostg", name=f"ost_{b}_{mc}_{nn}")
                nc.scalar.copy(o[:], pk[:])
                nc.gpsimd.tensor_add(o[:], o[:], xsl)
                nc.sync.dma_start(
                    out_ext[b, mc * P:(mc + 1) * P, nn * NT:(nn + 1) * NT], o[:])


_NC_CACHE = {}


def _get_nc():
    if "nc" not in _NC_CACHE:
        _NC_CACHE["nc"] = build()
    return _NC_CACHE["nc"]


def kernel(x, y, w_phi, w_theta, w_g, w_mask):
    x = np.ascontiguousarray(np.asarray(x, dtype=np.float32))
    y = np.ascontiguousarray(np.asarray(y, dtype=np.float32))
    bf = ml_dtypes.bfloat16
    wthT = np.ascontiguousarray(np.asarray(w_theta, np.float32).T).astype(bf)
    wgT = np.ascontiguousarray(np.asarray(w_g, np.float32).T).astype(bf)
    wphi = np.ascontiguousarray(np.asarray(w_phi, np.float32))
    wmkT = np.ascontiguousarray(np.asarray(w_mask, np.float32).T).astype(bf)

    nc = _get_nc()
    in_maps = []
    for c in range(NCORES):
        sl = slice(c * NB, (c + 1) * NB)
        in_maps.append({
            "x": x[sl].reshape(NB, C, N),
            "y": y[sl].reshape(NB, C, N),
            "wthT": wthT, "wgT": wgT, "wphi": wphi, "wmkT": wmkT,
        })
    res = run_bass_kernel_spmd(nc, in_maps, core_ids=list(range(NCORES)))
    out = np.concatenate([r["out"].reshape(NB, C, HH, WW) for r in res.results],
                         axis=0)
    return out
